# revision 13
# baseline (speedup 1.0000x reference)
"""Trainium2 Bass kernel for nn_MultiHeadAttention_37538014167348.

The reference einsum is 'bhqk,bhvd->bhqd' (k and v are independent), so the
attention output factorizes into (sum_k softmax_weights) * (sum_v V). Softmax
rows sum to exactly 1 (also true for the complex softmax), hence:

    out[b, q, :] = (sum_s x[b, s, :]) @ Wv + S * bv     (independent of q)

Q/K/mask/softmax drop out entirely. The kernel computes the row-sum of x, a
complex [1,768]x[768,768] matvec, and broadcasts the resulting row over the
1024 sequence positions.

Sharding over 8 cores: (batch b in 0..3) x (half of the 768 output features),
i.e. data parallel on B and tensor parallel across heads (6+6 of 12 heads).

Complex math is realized in f32: x stays interleaved (re,im) as [S, 2H]; the
weight matvec uses deinterleaved Re/Im planes of Wv (host-preshuffled to a
partition-major [128, 6*384] layout so the DMA is fully contiguous); outputs
are re/im planes re-assembled to complex64 on the host.

Per-core dataflow:
  1. x[b] arrives as 4 tiles [128, 3072] (partition p holds rows 2p, 2p+1).
  2. DVE tree-folds the 1024 rows down to deinterleaved tfa/tfb [128, 768]
     (Re/Im of partial column sums; 128 partial rows over s).
  3. 12 stationary matmuls (lhsT = tfa/tfb chunk [128,128], rhs = ones[128,1])
     finish the s-reduction across partitions, leaving u transposed in PSUM
     column form uta/utb [128, 6] -- no DRAM roundtrip transpose needed.
  4. Stage-2 matmuls use a replicated stationary (u column broadcast over all
     128 PE columns) so each accumulation lands PRE-BROADCAST as [128, 384]:
     re = a@C - b@D, im = a@D + b@C, in float32r (1 cycle/row).
  5. Bias rows are DMA-partition-broadcast, added on DVE, replicated 8x along
     free to [128, 3072] so each output plane is one contiguous 1.5MB DMA
     (partition p holds output rows 8p..8p+7).
"""

import os
import sys

import numpy as np

for _p in ("/opt/trn_rl_repo", "/root/.axon_site/_ro/trn_rl_repo"):
    if os.path.isdir(_p) and _p not in sys.path:
        sys.path.append(_p)

from concourse import bacc, mybir
from concourse.tile import TileContext
from concourse.bass_utils import run_bass_kernel_spmd

B, S, H = 4, 1024, 768
HALF = H // 2          # complex output columns per core
NCORES = 8
P = 128                # SBUF partitions
RPP = 2                # x rows packed per partition per tile
TW = 2 * H * RPP       # 3072 f32 per partition per x tile
NT = S // (P * RPP)    # 4 x tiles
KC = H // P            # 6 contraction chunks of 128
QR = S // P            # 8 output rows per partition
F32 = mybir.dt.float32
F32R = mybir.dt.float32r

_NC = None
LAST_RESULTS = None    # stashed BassKernelResults for profiling in test.py


def _build():
    nc = bacc.Bacc(None, target_bir_lowering=False)

    x = nc.dram_tensor("x", [S, 2 * H], F32, kind="ExternalInput")
    # host-preshuffled: cw[p, c*HALF+n] = Re(Wv)[c*128+p, half_cols[n]]
    cw = nc.dram_tensor("cw", [P, KC * HALF], F32, kind="ExternalInput")
    dw = nc.dram_tensor("dw", [P, KC * HALF], F32, kind="ExternalInput")
    brw = nc.dram_tensor("brw", [1, HALF], F32, kind="ExternalInput")  # Re(S*bv)
    biw = nc.dram_tensor("biw", [1, HALF], F32, kind="ExternalInput")  # Im(S*bv)
    out_re = nc.dram_tensor("out_re", [S, HALF], F32, kind="ExternalOutput")
    out_im = nc.dram_tensor("out_im", [S, HALF], F32, kind="ExternalOutput")

    # x rows s = t*256 + p*2 + r; partition p holds rows (2p, 2p+1) contiguously
    xv = x.rearrange("(t p r) f -> t p (r f)", t=NT, p=P, r=RPP)
    # output rows q = p*QR + r so each partition's 8 rows are contiguous 12KB
    ov_re = out_re.rearrange("(p q) n -> p (q n)", p=P, q=QR)
    ov_im = out_im.rearrange("(p q) n -> p (q n)", p=P, q=QR)

    with TileContext(nc) as tc:
        with tc.tile_pool(name="sbuf", bufs=1) as pool, \
             tc.tile_pool(name="psum", bufs=1, space="PSUM") as psum:

            ones = pool.tile([P, 1], F32)
            nc.vector.memset(ones[:], 1.0)

            # ---- weights (contiguous partition-major), chunked into 3 DMAs
            # per plane + f32r rounding casts on the scalar engine; the DMAs
            # are hard-ordered after x so x gets the full DMA bandwidth
            WCH = 2 * HALF  # 2 K-chunks per weight DMA
            c_sb = pool.tile([P, KC * HALF], F32)
            d_sb = pool.tile([P, KC * HALF], F32)
            c_r = pool.tile([P, KC * HALF], F32R)
            d_r = pool.tile([P, KC * HALF], F32R)
            wdmas = []
            for i in range(3):
                sl = slice(i * WCH, (i + 1) * WCH)
                wdmas.append(nc.scalar.dma_start(out=c_sb[:, sl], in_=cw[:, sl]))
                nc.scalar.mul(c_r[:, sl], c_sb[:, sl], 1.0)
                wdmas.append(nc.scalar.dma_start(out=d_sb[:, sl], in_=dw[:, sl]))
                nc.scalar.mul(d_r[:, sl], d_sb[:, sl], 1.0)
            brw_bc = pool.tile([P, HALF], F32)
            biw_bc = pool.tile([P, HALF], F32)
            nc.scalar.dma_start(out=brw_bc[:], in_=brw[:, :].to_broadcast([P, HALF]))
            nc.scalar.dma_start(out=biw_bc[:], in_=biw[:, :].to_broadcast([P, HALF]))

            # ---- stage 1: 8 half-tiles [128, 1536] (rows t*128..t*128+127);
            # each lands and is immediately folded into running accumulators
            # tfa/tfb via strided views (deinterleave re/im in the add)
            NT2 = S // P  # 8
            xdmas = []
            tfa = pool.tile([P, H], F32)
            tfb = pool.tile([P, H], F32)
            xv2 = x.rearrange("(t p) f -> t p f", t=NT2, p=P)
            for t in range(NT2):
                xt = pool.tile([P, 2 * H], F32, tag=f"x{t}")
                eng = nc.sync if t % 2 == 0 else nc.scalar
                xdmas.append(eng.dma_start(out=xt[:], in_=xv2[t]))
                vt = xt.rearrange("p (k t) -> p t k", t=2)
                if t == 0:
                    nc.vector.tensor_copy(tfa[:], vt[:, 0, :])
                    nc.vector.tensor_copy(tfb[:], vt[:, 1, :])
                else:
                    nc.vector.tensor_add(tfa[:], tfa[:], vt[:, 0, :])
                    nc.vector.tensor_add(tfb[:], tfb[:], vt[:, 1, :])

            # ---- finish s-reduction across partitions, output in column form:
            # uta[p, c] = Re(u)[c*128+p], utb = Im(u)
            uta = psum.tile([P, KC], F32)
            utb = psum.tile([P, KC], F32)
            for c in range(KC):
                nc.tensor.matmul(uta[:, c:c + 1], tfa[:, c * P:(c + 1) * P],
                                 ones[:], start=True, stop=True)
                nc.tensor.matmul(utb[:, c:c + 1], tfb[:, c * P:(c + 1) * P],
                                 ones[:], start=True, stop=True)

            # ---- stage 2: replicated-stationary matmuls accumulate the
            # complex matvec directly as a [128, 384] broadcast block
            bre = psum.tile([P, HALF], F32)
            bim = psum.tile([P, HALF], F32)
            rep_as, rep_bs, rep_bns = [], [], []
            for c in range(KC):
                rep_a = pool.tile([P, P], F32R, tag="rep_a", bufs=KC)
                rep_b = pool.tile([P, P], F32R, tag="rep_b", bufs=KC)
                rep_bn = pool.tile([P, P], F32R, tag="rep_bn", bufs=KC)
                nc.vector.tensor_copy(rep_a[:], uta[:, c:c + 1].to_broadcast([P, P]))
                nc.vector.tensor_copy(rep_b[:], utb[:, c:c + 1].to_broadcast([P, P]))
                nc.scalar.mul(rep_bn[:], utb[:, c:c + 1].to_broadcast([P, P]), -1.0)
                rep_as.append(rep_a)
                rep_bs.append(rep_b)
                rep_bns.append(rep_bn)
            # all re matmuls first so the re output plane can start its DMA
            # while the im plane is still accumulating
            for c in range(KC):
                cc = c_r[:, c * HALF:(c + 1) * HALF]
                dd = d_r[:, c * HALF:(c + 1) * HALF]
                nc.tensor.matmul(bre[:], rep_as[c][:], cc,
                                 start=(c == 0), stop=False)
                nc.tensor.matmul(bre[:], rep_bns[c][:], dd,
                                 start=False, stop=(c == KC - 1))
            for c in range(KC):
                cc = c_r[:, c * HALF:(c + 1) * HALF]
                dd = d_r[:, c * HALF:(c + 1) * HALF]
                nc.tensor.matmul(bim[:], rep_as[c][:], dd,
                                 start=(c == 0), stop=False)
                nc.tensor.matmul(bim[:], rep_bs[c][:], cc,
                                 start=False, stop=(c == KC - 1))

            # ---- bias add + replicate 8x along free for contiguous out DMA
            bc_re = pool.tile([P, HALF], F32)
            bc_im = pool.tile([P, HALF], F32)
            nc.vector.tensor_add(bc_re[:], bre[:], brw_bc[:])
            nc.vector.tensor_add(bc_im[:], bim[:], biw_bc[:])
            vr = bc_re[:].unsqueeze(1).to_broadcast([P, QR, HALF])
            vi = bc_im[:].unsqueeze(1).to_broadcast([P, QR, HALF])
            ovr = ov_re.rearrange("p (q n) -> p q n", q=QR)
            ovi = ov_im.rearrange("p (q n) -> p q n", q=QR)
            nc.sync.dma_start(out=ovr, in_=vr)
            nc.scalar.dma_start(out=ovi, in_=vi)

    nc.finalize()
    return nc


def _get_nc():
    global _NC
    if _NC is None:
        _NC = _build()
    return _NC


def _preshuffle(w_plane, j):
    # [768, 384] half -> [128, 6*384] with row k=c*128+p at (p, c*384..)
    half = w_plane[:, j * HALF:(j + 1) * HALF]           # [768, 384]
    return np.ascontiguousarray(
        half.reshape(KC, P, HALF).transpose(1, 0, 2).reshape(P, KC * HALF))


def make_in_maps(x, Wv, bv):
    xf = np.ascontiguousarray(x).view(np.float32).reshape(B, S, 2 * H)
    Wv = np.ascontiguousarray(Wv)
    bv = np.ascontiguousarray(bv)
    wre, wim = Wv.real.copy(), Wv.imag.copy()
    in_maps = []
    for core in range(NCORES):
        b, j = divmod(core, 2)
        cols = slice(j * HALF, (j + 1) * HALF)
        in_maps.append({
            "x": xf[b],
            "cw": _preshuffle(wre, j),
            "dw": _preshuffle(wim, j),
            "brw": np.ascontiguousarray(np.float32(S) * bv[cols].real)[None, :],
            "biw": np.ascontiguousarray(np.float32(S) * bv[cols].imag)[None, :],
        })
    return in_maps


def kernel(x, Wq, bq, Wk, bk, Wv, bv, mask, trace=False):
    global LAST_RESULTS
    in_maps = make_in_maps(np.asarray(x), np.asarray(Wv), np.asarray(bv))
    res = run_bass_kernel_spmd(_get_nc(), in_maps, core_ids=list(range(NCORES)),
                               trace=trace)
    LAST_RESULTS = res
    out = np.empty((B, S, H), dtype=np.complex64)
    for core in range(NCORES):
        b, j = divmod(core, 2)
        cols = slice(j * HALF, (j + 1) * HALF)
        r = res.results[core]
        out[b, :, cols] = r["out_re"] + 1j * r["out_im"]
    return out


# revision 14
# speedup vs baseline: 1.2553x; 1.2553x over previous
"""Trainium2 Bass kernel for nn_MultiHeadAttention_37538014167348.

The reference einsum is 'bhqk,bhvd->bhqd' (k and v are independent), so the
attention output factorizes into (sum_k softmax_weights) * (sum_v V). Softmax
rows sum to exactly 1 (also true for the complex softmax), hence:

    out[b, q, :] = (sum_s x[b, s, :]) @ Wv + S * bv     (independent of q)

Q/K/mask/softmax drop out entirely. The kernel computes the row-sum of x, a
complex [1,768]x[768,768] matvec, and broadcasts the resulting row over the
1024 sequence positions.

Sharding over 8 cores: (batch b in 0..3) x (half of the 768 output features),
i.e. data parallel on B and tensor parallel across heads (6+6 of 12 heads).

Complex math is realized in f32: x stays interleaved (re,im) as [S, 2H]; the
weight matvec uses deinterleaved Re/Im planes of Wv (host-preshuffled to a
partition-major [128, 6*384] layout so the DMA is fully contiguous); outputs
are re/im planes re-assembled to complex64 on the host.

Per-core dataflow:
  1. x[b] arrives as 4 tiles [128, 3072] (partition p holds rows 2p, 2p+1).
  2. DVE tree-folds the 1024 rows down to deinterleaved tfa/tfb [128, 768]
     (Re/Im of partial column sums; 128 partial rows over s).
  3. 12 stationary matmuls (lhsT = tfa/tfb chunk [128,128], rhs = ones[128,1])
     finish the s-reduction across partitions, leaving u transposed in PSUM
     column form uta/utb [128, 6] -- no DRAM roundtrip transpose needed.
  4. Stage-2 matmuls use a replicated stationary (u column broadcast over all
     128 PE columns) so each accumulation lands PRE-BROADCAST as [128, 384]:
     re = a@C - b@D, im = a@D + b@C, in float32r (1 cycle/row).
  5. Bias rows are DMA-partition-broadcast, added on DVE, replicated 8x along
     free to [128, 3072] so each output plane is one contiguous 1.5MB DMA
     (partition p holds output rows 8p..8p+7).
"""

import os
import sys

import numpy as np

for _p in ("/opt/trn_rl_repo", "/root/.axon_site/_ro/trn_rl_repo"):
    if os.path.isdir(_p) and _p not in sys.path:
        sys.path.append(_p)

from concourse import bacc, mybir
from concourse.tile import TileContext
from concourse.bass_utils import run_bass_kernel_spmd

B, S, H = 4, 1024, 768
HALF = H // 2          # complex output columns per core
NCORES = 8
P = 128                # SBUF partitions
RPP = 2                # x rows packed per partition per tile
TW = 2 * H * RPP       # 3072 f32 per partition per x tile
NT = S // (P * RPP)    # 4 x tiles
KC = H // P            # 6 contraction chunks of 128
QR = S // P            # 8 output rows per partition
F32 = mybir.dt.float32
F32R = mybir.dt.float32r

_NC = None
LAST_RESULTS = None    # stashed BassKernelResults for profiling in test.py


def _build():
    nc = bacc.Bacc(None, target_bir_lowering=False)

    x = nc.dram_tensor("x", [S, 2 * H], F32, kind="ExternalInput")
    # host-preshuffled: cw[p, c*HALF+n] = Re(Wv)[c*128+p, half_cols[n]]
    cw = nc.dram_tensor("cw", [P, KC * HALF], F32, kind="ExternalInput")
    dw = nc.dram_tensor("dw", [P, KC * HALF], F32, kind="ExternalInput")
    brw = nc.dram_tensor("brw", [1, HALF], F32, kind="ExternalInput")  # Re(S*bv)
    biw = nc.dram_tensor("biw", [1, HALF], F32, kind="ExternalInput")  # Im(S*bv)
    out_re = nc.dram_tensor("out_re", [S, HALF], F32, kind="ExternalOutput")
    out_im = nc.dram_tensor("out_im", [S, HALF], F32, kind="ExternalOutput")

    # x rows s = t*256 + p*2 + r; partition p holds rows (2p, 2p+1) contiguously
    xv = x.rearrange("(t p r) f -> t p (r f)", t=NT, p=P, r=RPP)
    # output rows q = p*QR + r so each partition's 8 rows are contiguous 12KB
    ov_re = out_re.rearrange("(p q) n -> p (q n)", p=P, q=QR)
    ov_im = out_im.rearrange("(p q) n -> p (q n)", p=P, q=QR)

    with TileContext(nc) as tc:
        with tc.tile_pool(name="sbuf", bufs=1) as pool, \
             tc.tile_pool(name="psum", bufs=1, space="PSUM") as psum:

            ones = pool.tile([P, 1], F32)
            nc.vector.memset(ones[:], 1.0)

            # ---- weights (contiguous partition-major), chunked into 3 DMAs
            # per plane + f32r rounding casts on the scalar engine; the DMAs
            # are hard-ordered after x so x gets the full DMA bandwidth
            WCH = 2 * HALF  # 2 K-chunks per weight DMA
            c_sb = pool.tile([P, KC * HALF], F32)
            d_sb = pool.tile([P, KC * HALF], F32)
            c_r = pool.tile([P, KC * HALF], F32R)
            d_r = pool.tile([P, KC * HALF], F32R)
            wdmas = []
            for i in range(3):
                sl = slice(i * WCH, (i + 1) * WCH)
                wdmas.append(nc.scalar.dma_start(out=c_sb[:, sl], in_=cw[:, sl]))
                nc.scalar.mul(c_r[:, sl], c_sb[:, sl], 1.0)
                wdmas.append(nc.scalar.dma_start(out=d_sb[:, sl], in_=dw[:, sl]))
                nc.scalar.mul(d_r[:, sl], d_sb[:, sl], 1.0)
            brw_bc = pool.tile([P, HALF], F32)
            biw_bc = pool.tile([P, HALF], F32)
            nc.scalar.dma_start(out=brw_bc[:], in_=brw[:, :].to_broadcast([P, HALF]))
            nc.scalar.dma_start(out=biw_bc[:], in_=biw[:, :].to_broadcast([P, HALF]))

            # ---- stage 1: 8 half-tiles [128, 1536] (rows t*128..t*128+127);
            # each lands and is immediately folded into running accumulators
            # tfa/tfb via strided views (deinterleave re/im in the add)
            NT2 = S // P  # 8
            xdmas = []
            tfa = pool.tile([P, H], F32)
            tfb = pool.tile([P, H], F32)
            xv2 = x.rearrange("(t p) f -> t p f", t=NT2, p=P)
            for t in range(NT2):
                xt = pool.tile([P, 2 * H], F32, tag=f"x{t}")
                xdmas.append(nc.sync.dma_start(out=xt[:], in_=xv2[t]))
                vt = xt.rearrange("p (k t) -> p t k", t=2)
                if t == 0:
                    nc.vector.tensor_copy(tfa[:], vt[:, 0, :])
                    nc.vector.tensor_copy(tfb[:], vt[:, 1, :])
                else:
                    nc.vector.tensor_add(tfa[:], tfa[:], vt[:, 0, :])
                    nc.vector.tensor_add(tfb[:], tfb[:], vt[:, 1, :])

            # ---- finish s-reduction across partitions, output in column form:
            # uta[p, c] = Re(u)[c*128+p], utb = Im(u)
            uta = psum.tile([P, KC], F32)
            utb = psum.tile([P, KC], F32)
            for c in range(KC):
                nc.tensor.matmul(uta[:, c:c + 1], tfa[:, c * P:(c + 1) * P],
                                 ones[:], start=True, stop=True)
                nc.tensor.matmul(utb[:, c:c + 1], tfb[:, c * P:(c + 1) * P],
                                 ones[:], start=True, stop=True)

            # ---- stage 2: replicated-stationary matmuls accumulate the
            # complex matvec directly as a [128, 384] broadcast block
            bre = psum.tile([P, HALF], F32)
            bim = psum.tile([P, HALF], F32)
            rep_as, rep_bs, rep_bns = [], [], []
            for c in range(KC):
                rep_a = pool.tile([P, P], F32R, tag="rep_a", bufs=KC)
                rep_b = pool.tile([P, P], F32R, tag="rep_b", bufs=KC)
                rep_bn = pool.tile([P, P], F32R, tag="rep_bn", bufs=KC)
                nc.vector.tensor_copy(rep_a[:], uta[:, c:c + 1].to_broadcast([P, P]))
                nc.vector.tensor_copy(rep_b[:], utb[:, c:c + 1].to_broadcast([P, P]))
                nc.scalar.mul(rep_bn[:], utb[:, c:c + 1].to_broadcast([P, P]), -1.0)
                rep_as.append(rep_a)
                rep_bs.append(rep_b)
                rep_bns.append(rep_bn)
            # all re matmuls first so the re output plane can start its DMA
            # while the im plane is still accumulating
            for c in range(KC):
                cc = c_r[:, c * HALF:(c + 1) * HALF]
                dd = d_r[:, c * HALF:(c + 1) * HALF]
                nc.tensor.matmul(bre[:], rep_as[c][:], cc,
                                 start=(c == 0), stop=False)
                nc.tensor.matmul(bre[:], rep_bns[c][:], dd,
                                 start=False, stop=(c == KC - 1))
            for c in range(KC):
                cc = c_r[:, c * HALF:(c + 1) * HALF]
                dd = d_r[:, c * HALF:(c + 1) * HALF]
                nc.tensor.matmul(bim[:], rep_as[c][:], dd,
                                 start=(c == 0), stop=False)
                nc.tensor.matmul(bim[:], rep_bs[c][:], cc,
                                 start=False, stop=(c == KC - 1))

            # ---- bias add + replicate 8x along free for contiguous out DMA
            bc_re = pool.tile([P, HALF], F32)
            bc_im = pool.tile([P, HALF], F32)
            nc.vector.tensor_add(bc_re[:], bre[:], brw_bc[:])
            nc.vector.tensor_add(bc_im[:], bim[:], biw_bc[:])
            vr = bc_re[:].unsqueeze(1).to_broadcast([P, QR, HALF])
            vi = bc_im[:].unsqueeze(1).to_broadcast([P, QR, HALF])
            ovr = ov_re.rearrange("p (q n) -> p q n", q=QR)
            ovi = ov_im.rearrange("p (q n) -> p q n", q=QR)
            nc.sync.dma_start(out=ovr, in_=vr)
            nc.scalar.dma_start(out=ovi, in_=vi)

    nc.finalize()
    return nc


def _get_nc():
    global _NC
    if _NC is None:
        _NC = _build()
    return _NC


def _preshuffle(w_plane, j):
    # [768, 384] half -> [128, 6*384] with row k=c*128+p at (p, c*384..)
    half = w_plane[:, j * HALF:(j + 1) * HALF]           # [768, 384]
    return np.ascontiguousarray(
        half.reshape(KC, P, HALF).transpose(1, 0, 2).reshape(P, KC * HALF))


def make_in_maps(x, Wv, bv):
    xf = np.ascontiguousarray(x).view(np.float32).reshape(B, S, 2 * H)
    Wv = np.ascontiguousarray(Wv)
    bv = np.ascontiguousarray(bv)
    wre, wim = Wv.real.copy(), Wv.imag.copy()
    in_maps = []
    for core in range(NCORES):
        b, j = divmod(core, 2)
        cols = slice(j * HALF, (j + 1) * HALF)
        in_maps.append({
            "x": xf[b],
            "cw": _preshuffle(wre, j),
            "dw": _preshuffle(wim, j),
            "brw": np.ascontiguousarray(np.float32(S) * bv[cols].real)[None, :],
            "biw": np.ascontiguousarray(np.float32(S) * bv[cols].imag)[None, :],
        })
    return in_maps


def kernel(x, Wq, bq, Wk, bk, Wv, bv, mask, trace=False):
    global LAST_RESULTS
    in_maps = make_in_maps(np.asarray(x), np.asarray(Wv), np.asarray(bv))
    res = run_bass_kernel_spmd(_get_nc(), in_maps, core_ids=list(range(NCORES)),
                               trace=trace)
    LAST_RESULTS = res
    out = np.empty((B, S, H), dtype=np.complex64)
    for core in range(NCORES):
        b, j = divmod(core, 2)
        cols = slice(j * HALF, (j + 1) * HALF)
        r = res.results[core]
        out[b, :, cols] = r["out_re"] + 1j * r["out_im"]
    return out
